# revision 18
# baseline (speedup 1.0000x reference)
"""CLUB loss kernel for Trainium2 (8 NeuronCores, SPMD).

Math
----
Reference computes, with flat_x = transpose(x,(0,2,3,1)).reshape(N,d),
ie = exp(-p_logvar):

  positive[i] = -0.5 * sum_d (x_i - mu_i)^2 * ie_i
  negative[i] = -0.5 * mean_j [ <x_j^2, ie_i> - 2 <x_j, mu_i*ie_i> + <mu_i^2, ie_i> ]
  loss = mean_i (positive - negative)

Because the loss only needs mean_j of a form affine in (x_j, x_j^2), the
(N,N) distance matrix collapses:

  sum_j D[i,j] = <ie_i, Sx2> - 2 <mu_i*ie_i, Sx> + N*<mu_i^2, ie_i>
  with Sx = sum_j x_j, Sx2 = sum_j x_j^2   (d-vectors, global over all rows)

So each core k (rows [784k, 784k+784) == batch element k) reduces its slab to
seven per-channel vectors (free-axis reductions in a channels-on-partitions
layout):

  A    = sum_i ie_i            B    = sum_i mu_i*ie_i
  Sx   = sum_i x_i             Sx2  = sum_i x_i^2
  Px2  = sum_i x_i^2*ie_i      Pxbm = sum_i x_i*mu_i*ie_i
  Cv   = sum_i mu_i^2*ie_i

and the host combines:
  P    = sum(Px2) - 2*sum(Pxbm) + sum(Cv)              # sum_i -2*positive[i]
  neg  = <A, Sx2_g> - 2 <B, Sx_g> + N*sum(Cv_g)        # sum_{i,j} D[i,j]
  loss = (-0.5*P + 0.5/N * neg) / N

Sharding: data-parallel over rows i; x arrives in DRAM already channels-major
per batch element (x[k] is (512, 784) == flat_x-slab transposed), so the
device kernel needs no transposes. mu/logvar slabs are transposed on the host
during input marshalling.
"""

import numpy as np

B, D, H, W = 8, 512, 28, 28
N = B * H * W            # 6272 rows
NCORES = 8
ROWS = N // NCORES       # 784 rows per core == H*W
NT = D // 128            # 4 channel tiles of 128 partitions
NCOLS = 4 * NT + 3 * (NT // 2)   # A,B,Sx,Sx2 per tile + Px2,Pxbm,Cv per pair

_PROGRAM_CACHE: dict = {}


def build_program():
    """One Bass program, broadcast SPMD to all 8 cores (data differs per core).

    Raw Bass (no TileContext): this walrus build rejects Tile's drain tail
    ("Too many sync wait commands") and the InstTensorTensorReduce encoding
    ("ISA wrong length"), so sync is managed manually and the fused
    multiply+reduce uses scalar_tensor_tensor (which encodes fine).
    """
    from contextlib import ExitStack

    import concourse.bass as bass
    import concourse.mybir as mybir

    f32 = mybir.dt.float32
    Alu = mybir.AluOpType
    Act = mybir.ActivationFunctionType

    nc = bass.Bass()
    xT = nc.declare_dram_parameter("xT", [D, ROWS], f32, isOutput=False)
    muT = nc.declare_dram_parameter("muT", [D, ROWS], f32, isOutput=False)
    lvT = nc.declare_dram_parameter("lvT", [D, ROWS], f32, isOutput=False)
    stats = nc.declare_dram_parameter("stats", [128, NCOLS], f32, isOutput=True)

    PAIR = 2 * ROWS  # two channel-tiles side by side in the free dim

    with ExitStack() as ctx:
        sb = lambda name, shape: ctx.enter_context(nc.sbuf_tensor(name, shape, f32))
        # paired tensors: pair p holds tiles 2p, 2p+1 as column halves
        xp = [sb(f"xp{p}", [128, PAIR]) for p in range(NT // 2)]
        mup = [sb(f"mup{p}", [128, PAIR]) for p in range(NT // 2)]
        iep = [sb(f"iep{p}", [128, PAIR]) for p in range(NT // 2)]
        xsp = [sb(f"xsp{p}", [128, PAIR]) for p in range(NT // 2)]
        bmp = [sb(f"bmp{p}", [128, PAIR]) for p in range(NT // 2)]
        lv = [sb(f"lv{t}", [128, ROWS]) for t in range(NT)]
        wt = sb("wt", [128, 1])              # ACT table warm-up scratch
        sc_act = sb("sc_act", [128, ROWS])   # dead ACT outs (ACT is in-order)
        sc_dve = sb("sc_dve", [128, PAIR])   # dead DVE outs (DVE is in-order)
        st = sb("st", [128, NCOLS])

        def half(tensor_list, t):
            return tensor_list[t // 2][:, (t % 2) * ROWS : (t % 2 + 1) * ROWS]

        sem_x = [ctx.enter_context(nc.semaphore(f"sx{t}")) for t in range(NT)]
        sem_mu = [ctx.enter_context(nc.semaphore(f"sm{t}")) for t in range(NT)]
        sem_lv = [ctx.enter_context(nc.semaphore(f"sl{t}")) for t in range(NT)]
        act_sem = ctx.enter_context(nc.semaphore("act"))
        dve_sem = ctx.enter_context(nc.semaphore("dve"))
        out_sem = ctx.enter_context(nc.semaphore("outs"))
        block = ctx.enter_context(nc.Block())

        # column layout: A[t]=t, B[t]=NT+t, Sx[t]=2NT+t, Sx2[t]=3NT+t,
        # Px2[p]=4NT+p, Pxbm[p]=4NT+NP+p, Cv[p]=4NT+2NP+p
        NP = NT // 2

        def col(c):
            return st[:, c : c + 1]

        @block.sync
        def _(sync):
            # HWDGE lv/mu DMAs on the otherwise-idle SP engine (lv first: ACT's
            # exp is the critical-path head).
            for t in range(NT):
                sl_ = slice(128 * t, 128 * (t + 1))
                sync.dma_start(lv[t][:], lvT[sl_, :]).then_inc(sem_lv[t], 16)
                sync.dma_start(half(mup, t), muT[sl_, :]).then_inc(sem_mu[t], 16)
            sync.wait_ge(act_sem, 9 + NT)
            sync.wait_ge(dve_sem, 2 * NT + NP)
            sync.dma_start(stats[:, :], st[:]).then_inc(out_sem, 16)
            sync.wait_ge(out_sem, 16)

        @block.gpsimd
        def _(gpsimd):
            # x DMAs issue in parallel on the SWDGE path.
            for t in range(NT):
                sl_ = slice(128 * t, 128 * (t + 1))
                gpsimd.dma_start(half(xp, t), xT[sl_, :]).then_inc(sem_x[t], 16)

        @block.scalar
        def _(scalar):
            # Dummy exp: hoists the ACT table load into the DMA-wait window.
            nc.scalar.activation(wt[:], wt[:], Act.Exp, bias=0.0, scale=0.0
                                 ).then_inc(act_sem, 1)
            for t in range(NT):
                scalar.wait_ge(sem_lv[t], 16)
                # ie = exp(-lv); accum -> A      (op 2t+2 on act_sem)
                nc.scalar.activation(half(iep, t), lv[t][:], Act.Exp, bias=0.0,
                                     scale=-1.0, accum_out=col(t)
                                     ).then_inc(act_sem, 1)
                scalar.wait_ge(sem_x[t], 16)
                # xs = x^2; accum -> Sx2         (op 2t+3 on act_sem)
                nc.scalar.activation(half(xsp, t), half(xp, t), Act.Square,
                                     accum_out=col(3 * NT + t)).then_inc(act_sem, 1)
            for t in range(NT):
                if t > 0:
                    scalar.wait_ge(act_sem, 9 + t)   # sc_act WAW (self, trivial)
                # copy of x (dead); accum -> Sx  (op 10+t)
                nc.scalar.activation(sc_act[:], half(xp, t), Act.Copy,
                                     accum_out=col(2 * NT + t)).then_inc(act_sem, 1)

        @block.vector
        def _(vector):
            def bmst(t):   # bm_t = mu_t*ie_t; accum -> B[t]
                vector.wait_ge(act_sem, 2 * t + 2)   # ie[t] ready
                vector.wait_ge(sem_mu[t], 16)
                nc.vector.scalar_tensor_tensor(
                    half(bmp, t), half(mup, t), 1.0, half(iep, t),
                    Alu.mult, Alu.mult, accum_out=col(NT + t)).then_inc(dve_sem, 1)

            def pairst(in0p, in1p, p, c, act_need, dve_need):
                if act_need:
                    vector.wait_ge(act_sem, act_need)
                if dve_need:
                    vector.wait_ge(dve_sem, dve_need)
                nc.vector.scalar_tensor_tensor(
                    sc_dve[:], in0p[p][:], 1.0, in1p[p][:], Alu.mult, Alu.mult,
                    accum_out=col(c)).then_inc(dve_sem, 1)

            # DVE order interleaves pair ops with BMs to avoid stalls:
            # BM0(1) BM1(2) Px2p0(3) BM2(4) Pxbmp0(5) BM3(6) Cvp0(7)
            # Px2p1(8) Pxbmp1(9) Cvp1(10)
            bmst(0)
            bmst(1)
            pairst(xsp, iep, 0, 4 * NT + 0, act_need=5, dve_need=None)  # Px2 p0
            bmst(2)
            vector.wait_ge(sem_x[0], 16)
            vector.wait_ge(sem_x[1], 16)
            pairst(xp, bmp, 0, 4 * NT + NP + 0, act_need=None, dve_need=3)  # Pxbm p0
            bmst(3)
            pairst(mup, bmp, 0, 4 * NT + 2 * NP + 0, act_need=None, dve_need=5)  # Cv p0
            pairst(xsp, iep, 1, 4 * NT + 1, act_need=9, dve_need=7)  # Px2 p1
            vector.wait_ge(sem_x[2], 16)
            vector.wait_ge(sem_x[3], 16)
            pairst(xp, bmp, 1, 4 * NT + NP + 1, act_need=None, dve_need=8)  # Pxbm p1
            pairst(mup, bmp, 1, 4 * NT + 2 * NP + 1, act_need=None, dve_need=9)  # Cv p1

    return nc


def get_program():
    if "nc" not in _PROGRAM_CACHE:
        _PROGRAM_CACHE["nc"] = build_program()
    return _PROGRAM_CACHE["nc"]


def make_in_maps(x, p_mu, p_logvar):
    """Shard full inputs into per-core input maps (data-parallel over rows)."""
    x = np.asarray(x, dtype=np.float32)
    p_mu = np.asarray(p_mu, dtype=np.float32)
    p_logvar = np.asarray(p_logvar, dtype=np.float32)
    xk = x.reshape(NCORES, D, ROWS)  # core k's slab of flat_x, transposed
    in_maps = []
    for k in range(NCORES):
        rows = slice(ROWS * k, ROWS * (k + 1))
        in_maps.append({
            "xT": np.ascontiguousarray(xk[k]),
            "muT": np.ascontiguousarray(p_mu[rows].T),
            "lvT": np.ascontiguousarray(p_logvar[rows].T),
        })
    return in_maps


def _unpack_stats(stats_arr):
    """(128, NCOLS) device layout -> (A, B, Sx, Sx2) as (512,) + 3 scalars."""
    a = stats_arr.astype(np.float64)
    vecs = [a[:, s * NT : (s + 1) * NT].T.reshape(D) for s in range(4)]  # t-major
    px2 = a[:, 4 * NT : 4 * NT + NT // 2].sum()
    pxbm = a[:, 4 * NT + NT // 2 : 4 * NT + NT].sum()
    cv = a[:, 4 * NT + NT : 4 * NT + 3 * (NT // 2)].sum()
    return vecs, (px2, pxbm, cv)


def combine(stats_per_core):
    """Host epilogue: all-reduce the per-core stat vectors and form the scalar."""
    A = np.zeros(D); Bv = np.zeros(D); Sx = np.zeros(D); Sx2 = np.zeros(D)
    Px2 = Pxbm = Csum = 0.0
    for arr in stats_per_core:
        (a, b, sx, sx2), (px2, pxbm, cv) = _unpack_stats(arr)
        A += a; Bv += b; Sx += sx; Sx2 += sx2
        Px2 += px2; Pxbm += pxbm; Csum += cv
    P = Px2 - 2.0 * Pxbm + Csum                   # sum_i sum_d (x-mu)^2*ie
    neg = A @ Sx2 - 2.0 * (Bv @ Sx) + N * Csum    # sum_{i,j} D[i,j]
    loss = (-0.5 * P + 0.5 / N * neg) / N
    return np.float32(loss)


def run_on_device(in_maps, trace=False, **kwargs):
    from concourse.bass_utils import run_bass_kernel_spmd

    nc = get_program()
    return run_bass_kernel_spmd(nc, in_maps, list(range(NCORES)), trace=trace,
                                **kwargs)


def kernel(x, p_mu, p_logvar):
    in_maps = make_in_maps(x, p_mu, p_logvar)
    br = run_on_device(in_maps)
    return combine([r["stats"] for r in br.results])


# revision 21
# speedup vs baseline: 1.0446x; 1.0446x over previous
"""CLUB loss kernel for Trainium2 (8 NeuronCores, SPMD).

Math
----
Reference computes, with flat_x = transpose(x,(0,2,3,1)).reshape(N,d),
ie = exp(-p_logvar):

  positive[i] = -0.5 * sum_d (x_i - mu_i)^2 * ie_i
  negative[i] = -0.5 * mean_j [ <x_j^2, ie_i> - 2 <x_j, mu_i*ie_i> + <mu_i^2, ie_i> ]
  loss = mean_i (positive - negative)

Because the loss only needs mean_j of a form affine in (x_j, x_j^2), the
(N,N) distance matrix collapses:

  sum_j D[i,j] = <ie_i, Sx2> - 2 <mu_i*ie_i, Sx> + N*<mu_i^2, ie_i>
  with Sx = sum_j x_j, Sx2 = sum_j x_j^2   (d-vectors, global over all rows)

So each core k (rows [784k, 784k+784) == batch element k) reduces its slab to
seven per-channel vectors (free-axis reductions in a channels-on-partitions
layout):

  A    = sum_i ie_i            B    = sum_i mu_i*ie_i
  Sx   = sum_i x_i             Sx2  = sum_i x_i^2
  Px2  = sum_i x_i^2*ie_i      Pxbm = sum_i x_i*mu_i*ie_i
  Cv   = sum_i mu_i^2*ie_i

and the host combines:
  P    = sum(Px2) - 2*sum(Pxbm) + sum(Cv)              # sum_i -2*positive[i]
  neg  = <A, Sx2_g> - 2 <B, Sx_g> + N*sum(Cv_g)        # sum_{i,j} D[i,j]
  loss = (-0.5*P + 0.5/N * neg) / N

Sharding: data-parallel over rows i; x arrives in DRAM already channels-major
per batch element (x[k] is (512, 784) == flat_x-slab transposed), so the
device kernel needs no transposes. mu/logvar slabs are transposed on the host
during input marshalling.
"""

import numpy as np

B, D, H, W = 8, 512, 28, 28
N = B * H * W            # 6272 rows
NCORES = 8
ROWS = N // NCORES       # 784 rows per core == H*W
NT = D // 128            # 4 channel tiles of 128 partitions
NCOLS = 7 * NT           # A,B,Sx,Sx2,Px2,Pxbm,Cv per channel tile

_PROGRAM_CACHE: dict = {}


def build_program():
    """One Bass program, broadcast SPMD to all 8 cores (data differs per core).

    Raw Bass (no TileContext): this walrus build rejects Tile's drain tail
    ("Too many sync wait commands") and the InstTensorTensorReduce encoding
    ("ISA wrong length"), so sync is managed manually and the fused
    multiply+reduce uses scalar_tensor_tensor (which encodes fine).
    """
    from contextlib import ExitStack

    import concourse.bass as bass
    import concourse.mybir as mybir

    f32 = mybir.dt.float32
    Alu = mybir.AluOpType
    Act = mybir.ActivationFunctionType

    nc = bass.Bass()
    xT = nc.declare_dram_parameter("xT", [D, ROWS], f32, isOutput=False)
    muT = nc.declare_dram_parameter("muT", [D, ROWS], f32, isOutput=False)
    lvT = nc.declare_dram_parameter("lvT", [D, ROWS], f32, isOutput=False)
    stats = nc.declare_dram_parameter("stats", [128, NCOLS], f32, isOutput=True)

    with ExitStack() as ctx:
        sb = lambda name, shape: ctx.enter_context(nc.sbuf_tensor(name, shape, f32))
        x = [sb(f"x{t}", [128, ROWS]) for t in range(NT)]
        mu = [sb(f"mu{t}", [128, ROWS]) for t in range(NT)]
        lv = [sb(f"lv{t}", [128, ROWS]) for t in range(NT)]
        ie = [sb(f"ie{t}", [128, ROWS]) for t in range(NT)]
        xs = [sb(f"xs{t}", [128, ROWS]) for t in range(NT)]
        bm = [sb(f"bm{t}", [128, ROWS]) for t in range(NT)]
        wt = sb("wt", [128, 1])              # ACT table warm-up scratch
        sc_act = sb("sc_act", [128, ROWS])   # dead ACT outs (ACT is in-order)
        sc_dve = sb("sc_dve", [128, ROWS])   # dead DVE outs (DVE is in-order)
        st = sb("st", [128, NCOLS])

        sem_x = [ctx.enter_context(nc.semaphore(f"sx{t}")) for t in range(NT)]
        sem_mu = [ctx.enter_context(nc.semaphore(f"sm{t}")) for t in range(NT)]
        sem_lv = [ctx.enter_context(nc.semaphore(f"sl{t}")) for t in range(NT)]
        act_sem = ctx.enter_context(nc.semaphore("act"))
        dve_sem = ctx.enter_context(nc.semaphore("dve"))
        out_sem = ctx.enter_context(nc.semaphore("outs"))
        block = ctx.enter_context(nc.Block())

        # column layout: A[t]=t, B[t]=NT+t, Sx[t]=2NT+t, Sx2[t]=3NT+t,
        # Px2[t]=4NT+t, Pxbm[t]=5NT+t, Cv[t]=6NT+t  (Px2/Pxbm/Cv host-summed)
        def col(c):
            return st[:, c : c + 1]

        @block.sync
        def _(sync):
            # HWDGE lv/mu DMAs, tile-major, on the otherwise-idle SP engine.
            for t in range(NT):
                sl_ = slice(128 * t, 128 * (t + 1))
                sync.dma_start(lv[t][:], lvT[sl_, :]).then_inc(sem_lv[t], 16)
                sync.dma_start(mu[t][:], muT[sl_, :]).then_inc(sem_mu[t], 16)
            sync.wait_ge(act_sem, 3 * NT + 1)
            sync.wait_ge(dve_sem, 4 * NT)
            sync.dma_start(stats[:, :], st[:]).then_inc(out_sem, 16)
            sync.wait_ge(out_sem, 16)

        @block.gpsimd
        def _(gpsimd):
            # x DMAs stream in parallel on the SWDGE path.
            for t in range(NT):
                sl_ = slice(128 * t, 128 * (t + 1))
                gpsimd.dma_start(x[t][:], xT[sl_, :]).then_inc(sem_x[t], 16)

        @block.scalar
        def _(scalar):
            # Dummy exp: hoists the ACT table load into the DMA-wait window.
            nc.scalar.activation(wt[:], wt[:], Act.Exp, bias=0.0, scale=0.0
                                 ).then_inc(act_sem, 1)
            for t in range(NT):
                scalar.wait_ge(sem_lv[t], 16)
                # ie = exp(-lv); accum -> A      (act op 3t+2)
                nc.scalar.activation(ie[t][:], lv[t][:], Act.Exp, bias=0.0,
                                     scale=-1.0, accum_out=col(t)
                                     ).then_inc(act_sem, 1)
                scalar.wait_ge(sem_x[t], 16)
                # xs = x^2; accum -> Sx2         (act op 3t+3)
                nc.scalar.activation(xs[t][:], x[t][:], Act.Square,
                                     accum_out=col(3 * NT + t)).then_inc(act_sem, 1)
                if t > 0:
                    scalar.wait_ge(act_sem, 3 * t + 1)   # sc_act WAW (self)
                # copy of x (dead); accum -> Sx  (act op 3t+4)
                nc.scalar.activation(sc_act[:], x[t][:], Act.Copy,
                                     accum_out=col(2 * NT + t)).then_inc(act_sem, 1)

        @block.vector
        def _(vector):
            for t in range(NT):
                vector.wait_ge(act_sem, 3 * t + 2)   # ie[t] ready
                vector.wait_ge(sem_mu[t], 16)
                # bm = mu*ie; accum -> B[t]           (dve op 4t+1)
                nc.vector.scalar_tensor_tensor(
                    bm[t][:], mu[t][:], 1.0, ie[t][:], Alu.mult, Alu.mult,
                    accum_out=col(NT + t)).then_inc(dve_sem, 1)
                vector.wait_ge(act_sem, 3 * t + 3)   # xs[t] (and x[t]) ready
                if t > 0:
                    vector.wait_ge(dve_sem, 4 * t)   # sc_dve WAW (self)
                # xs*ie (dead); accum -> Px2[t]       (dve op 4t+2)
                nc.vector.scalar_tensor_tensor(
                    sc_dve[:], xs[t][:], 1.0, ie[t][:], Alu.mult, Alu.mult,
                    accum_out=col(4 * NT + t)).then_inc(dve_sem, 1)
                vector.wait_ge(dve_sem, 4 * t + 2)   # bm[t] RAW + sc_dve WAW
                # x*bm (dead); accum -> Pxbm[t]       (dve op 4t+3)
                nc.vector.scalar_tensor_tensor(
                    sc_dve[:], x[t][:], 1.0, bm[t][:], Alu.mult, Alu.mult,
                    accum_out=col(5 * NT + t)).then_inc(dve_sem, 1)
                vector.wait_ge(dve_sem, 4 * t + 3)   # sc_dve WAW (self)
                # mu*bm (dead); accum -> Cv[t]        (dve op 4t+4)
                nc.vector.scalar_tensor_tensor(
                    sc_dve[:], mu[t][:], 1.0, bm[t][:], Alu.mult, Alu.mult,
                    accum_out=col(6 * NT + t)).then_inc(dve_sem, 1)

    return nc


def get_program():
    if "nc" not in _PROGRAM_CACHE:
        _PROGRAM_CACHE["nc"] = build_program()
    return _PROGRAM_CACHE["nc"]


def make_in_maps(x, p_mu, p_logvar):
    """Shard full inputs into per-core input maps (data-parallel over rows)."""
    x = np.asarray(x, dtype=np.float32)
    p_mu = np.asarray(p_mu, dtype=np.float32)
    p_logvar = np.asarray(p_logvar, dtype=np.float32)
    xk = x.reshape(NCORES, D, ROWS)  # core k's slab of flat_x, transposed
    in_maps = []
    for k in range(NCORES):
        rows = slice(ROWS * k, ROWS * (k + 1))
        in_maps.append({
            "xT": np.ascontiguousarray(xk[k]),
            "muT": np.ascontiguousarray(p_mu[rows].T),
            "lvT": np.ascontiguousarray(p_logvar[rows].T),
        })
    return in_maps


def _unpack_stats(stats_arr):
    """(128, NCOLS) device layout -> (A, B, Sx, Sx2) as (512,) + 3 scalars."""
    a = stats_arr.astype(np.float64)
    vecs = [a[:, s * NT : (s + 1) * NT].T.reshape(D) for s in range(4)]  # t-major
    px2 = a[:, 4 * NT : 5 * NT].sum()
    pxbm = a[:, 5 * NT : 6 * NT].sum()
    cv = a[:, 6 * NT : 7 * NT].sum()
    return vecs, (px2, pxbm, cv)


def combine(stats_per_core):
    """Host epilogue: all-reduce the per-core stat vectors and form the scalar."""
    A = np.zeros(D); Bv = np.zeros(D); Sx = np.zeros(D); Sx2 = np.zeros(D)
    Px2 = Pxbm = Csum = 0.0
    for arr in stats_per_core:
        (a, b, sx, sx2), (px2, pxbm, cv) = _unpack_stats(arr)
        A += a; Bv += b; Sx += sx; Sx2 += sx2
        Px2 += px2; Pxbm += pxbm; Csum += cv
    P = Px2 - 2.0 * Pxbm + Csum                   # sum_i sum_d (x-mu)^2*ie
    neg = A @ Sx2 - 2.0 * (Bv @ Sx) + N * Csum    # sum_{i,j} D[i,j]
    loss = (-0.5 * P + 0.5 / N * neg) / N
    return np.float32(loss)


def run_on_device(in_maps, trace=False, **kwargs):
    from concourse.bass_utils import run_bass_kernel_spmd

    nc = get_program()
    return run_bass_kernel_spmd(nc, in_maps, list(range(NCORES)), trace=trace,
                                **kwargs)


def kernel(x, p_mu, p_logvar):
    in_maps = make_in_maps(x, p_mu, p_logvar)
    br = run_on_device(in_maps)
    return combine([r["stats"] for r in br.results])


# revision 22
# speedup vs baseline: 1.0947x; 1.0480x over previous
"""CLUB loss kernel for Trainium2 (8 NeuronCores, SPMD).

Math
----
Reference computes, with flat_x = transpose(x,(0,2,3,1)).reshape(N,d),
ie = exp(-p_logvar):

  positive[i] = -0.5 * sum_d (x_i - mu_i)^2 * ie_i
  negative[i] = -0.5 * mean_j [ <x_j^2, ie_i> - 2 <x_j, mu_i*ie_i> + <mu_i^2, ie_i> ]
  loss = mean_i (positive - negative)

Because the loss only needs mean_j of a form affine in (x_j, x_j^2), the
(N,N) distance matrix collapses:

  sum_j D[i,j] = <ie_i, Sx2> - 2 <mu_i*ie_i, Sx> + N*<mu_i^2, ie_i>
  with Sx = sum_j x_j, Sx2 = sum_j x_j^2   (d-vectors, global over all rows)

So each core k (rows [784k, 784k+784) == batch element k) reduces its slab to
seven per-channel vectors (free-axis reductions in a channels-on-partitions
layout):

  A    = sum_i ie_i            B    = sum_i mu_i*ie_i
  Sx   = sum_i x_i             Sx2  = sum_i x_i^2
  Px2  = sum_i x_i^2*ie_i      Pxbm = sum_i x_i*mu_i*ie_i
  Cv   = sum_i mu_i^2*ie_i

and the host combines:
  P    = sum(Px2) - 2*sum(Pxbm) + sum(Cv)              # sum_i -2*positive[i]
  neg  = <A, Sx2_g> - 2 <B, Sx_g> + N*sum(Cv_g)        # sum_{i,j} D[i,j]
  loss = (-0.5*P + 0.5/N * neg) / N

Sharding: data-parallel over rows i; x arrives in DRAM already channels-major
per batch element (x[k] is (512, 784) == flat_x-slab transposed), so the
device kernel needs no transposes. mu/logvar slabs are transposed on the host
during input marshalling.
"""

import numpy as np

B, D, H, W = 8, 512, 28, 28
N = B * H * W            # 6272 rows
NCORES = 8
ROWS = N // NCORES       # 784 rows per core == H*W
NT = D // 128            # 4 channel tiles of 128 partitions
NCOLS = 7 * NT           # A,B,Sx,Sx2,Px2,Pxbm,Cv per channel tile

_PROGRAM_CACHE: dict = {}


def build_program():
    """One Bass program, broadcast SPMD to all 8 cores (data differs per core).

    Raw Bass (no TileContext): this walrus build rejects Tile's drain tail
    ("Too many sync wait commands") and the InstTensorTensorReduce encoding
    ("ISA wrong length"), so sync is managed manually and the fused
    multiply+reduce uses scalar_tensor_tensor (which encodes fine).
    """
    from contextlib import ExitStack

    import concourse.bass as bass
    import concourse.mybir as mybir

    f32 = mybir.dt.float32
    Alu = mybir.AluOpType
    Act = mybir.ActivationFunctionType

    nc = bass.Bass()
    xT = nc.declare_dram_parameter("xT", [D, ROWS], f32, isOutput=False)
    muT = nc.declare_dram_parameter("muT", [D, ROWS], f32, isOutput=False)
    lvT = nc.declare_dram_parameter("lvT", [D, ROWS], f32, isOutput=False)
    stats = nc.declare_dram_parameter("stats", [128, NCOLS], f32, isOutput=True)

    with ExitStack() as ctx:
        sb = lambda name, shape: ctx.enter_context(nc.sbuf_tensor(name, shape, f32))
        x = [sb(f"x{t}", [128, ROWS]) for t in range(NT)]
        mu = [sb(f"mu{t}", [128, ROWS]) for t in range(NT)]
        lv = [sb(f"lv{t}", [128, ROWS]) for t in range(NT)]
        ie = [sb(f"ie{t}", [128, ROWS]) for t in range(NT)]
        xs = [sb(f"xs{t}", [128, ROWS]) for t in range(NT)]
        bm = [sb(f"bm{t}", [128, ROWS]) for t in range(NT)]
        wt = sb("wt", [128, 1])              # ACT table warm-up scratch
        sc_act = sb("sc_act", [128, ROWS])   # dead ACT outs (ACT is in-order)
        sc_dve = sb("sc_dve", [128, ROWS])   # dead DVE outs (DVE is in-order)
        st = sb("st", [128, NCOLS])

        sem_x = [ctx.enter_context(nc.semaphore(f"sx{t}")) for t in range(NT)]
        sem_mu = [ctx.enter_context(nc.semaphore(f"sm{t}")) for t in range(NT)]
        sem_lv = [ctx.enter_context(nc.semaphore(f"sl{t}")) for t in range(NT)]
        act_sem = ctx.enter_context(nc.semaphore("act"))
        dve_sem = ctx.enter_context(nc.semaphore("dve"))
        out_sem = ctx.enter_context(nc.semaphore("outs"))
        block = ctx.enter_context(nc.Block())

        # column layout: A[t]=t, B[t]=NT+t, Sx[t]=2NT+t, Sx2[t]=3NT+t,
        # Px2[t]=4NT+t, Pxbm[t]=5NT+t, Cv[t]=6NT+t  (Px2/Pxbm/Cv host-summed)
        def col(c):
            return st[:, c : c + 1]

        @block.sync
        def _(sync):
            # All input DMAs on one HWDGE stream, ordered exactly in
            # consumption order (the DMA pipe is the roofline; arrival
            # order is everything).
            for t in range(NT):
                sl_ = slice(128 * t, 128 * (t + 1))
                sync.dma_start(lv[t][:], lvT[sl_, :]).then_inc(sem_lv[t], 16)
                sync.dma_start(mu[t][:], muT[sl_, :]).then_inc(sem_mu[t], 16)
                sync.dma_start(x[t][:], xT[sl_, :]).then_inc(sem_x[t], 16)
            sync.wait_ge(act_sem, 3 * NT + 1)
            sync.wait_ge(dve_sem, 4 * NT)
            sync.dma_start(stats[:, :], st[:]).then_inc(out_sem, 16)
            sync.wait_ge(out_sem, 16)

        @block.scalar
        def _(scalar):
            # Dummy exp: hoists the ACT table load into the DMA-wait window.
            nc.scalar.activation(wt[:], wt[:], Act.Exp, bias=0.0, scale=0.0
                                 ).then_inc(act_sem, 1)
            for t in range(NT):
                scalar.wait_ge(sem_lv[t], 16)
                # ie = exp(-lv); accum -> A      (act op 3t+2)
                nc.scalar.activation(ie[t][:], lv[t][:], Act.Exp, bias=0.0,
                                     scale=-1.0, accum_out=col(t)
                                     ).then_inc(act_sem, 1)
                scalar.wait_ge(sem_x[t], 16)
                # xs = x^2; accum -> Sx2         (act op 3t+3)
                nc.scalar.activation(xs[t][:], x[t][:], Act.Square,
                                     accum_out=col(3 * NT + t)).then_inc(act_sem, 1)
                if t > 0:
                    scalar.wait_ge(act_sem, 3 * t + 1)   # sc_act WAW (self)
                # copy of x (dead); accum -> Sx  (act op 3t+4)
                nc.scalar.activation(sc_act[:], x[t][:], Act.Copy,
                                     accum_out=col(2 * NT + t)).then_inc(act_sem, 1)

        @block.vector
        def _(vector):
            for t in range(NT):
                vector.wait_ge(act_sem, 3 * t + 2)   # ie[t] ready
                vector.wait_ge(sem_mu[t], 16)
                # bm = mu*ie; accum -> B[t]           (dve op 4t+1)
                nc.vector.scalar_tensor_tensor(
                    bm[t][:], mu[t][:], 1.0, ie[t][:], Alu.mult, Alu.mult,
                    accum_out=col(NT + t)).then_inc(dve_sem, 1)
                vector.wait_ge(act_sem, 3 * t + 3)   # xs[t] (and x[t]) ready
                if t > 0:
                    vector.wait_ge(dve_sem, 4 * t)   # sc_dve WAW (self)
                # xs*ie (dead); accum -> Px2[t]       (dve op 4t+2)
                nc.vector.scalar_tensor_tensor(
                    sc_dve[:], xs[t][:], 1.0, ie[t][:], Alu.mult, Alu.mult,
                    accum_out=col(4 * NT + t)).then_inc(dve_sem, 1)
                vector.wait_ge(dve_sem, 4 * t + 2)   # bm[t] RAW + sc_dve WAW
                # x*bm (dead); accum -> Pxbm[t]       (dve op 4t+3)
                nc.vector.scalar_tensor_tensor(
                    sc_dve[:], x[t][:], 1.0, bm[t][:], Alu.mult, Alu.mult,
                    accum_out=col(5 * NT + t)).then_inc(dve_sem, 1)
                vector.wait_ge(dve_sem, 4 * t + 3)   # sc_dve WAW (self)
                # mu*bm (dead); accum -> Cv[t]        (dve op 4t+4)
                nc.vector.scalar_tensor_tensor(
                    sc_dve[:], mu[t][:], 1.0, bm[t][:], Alu.mult, Alu.mult,
                    accum_out=col(6 * NT + t)).then_inc(dve_sem, 1)

    return nc


def get_program():
    if "nc" not in _PROGRAM_CACHE:
        _PROGRAM_CACHE["nc"] = build_program()
    return _PROGRAM_CACHE["nc"]


def make_in_maps(x, p_mu, p_logvar):
    """Shard full inputs into per-core input maps (data-parallel over rows)."""
    x = np.asarray(x, dtype=np.float32)
    p_mu = np.asarray(p_mu, dtype=np.float32)
    p_logvar = np.asarray(p_logvar, dtype=np.float32)
    xk = x.reshape(NCORES, D, ROWS)  # core k's slab of flat_x, transposed
    in_maps = []
    for k in range(NCORES):
        rows = slice(ROWS * k, ROWS * (k + 1))
        in_maps.append({
            "xT": np.ascontiguousarray(xk[k]),
            "muT": np.ascontiguousarray(p_mu[rows].T),
            "lvT": np.ascontiguousarray(p_logvar[rows].T),
        })
    return in_maps


def _unpack_stats(stats_arr):
    """(128, NCOLS) device layout -> (A, B, Sx, Sx2) as (512,) + 3 scalars."""
    a = stats_arr.astype(np.float64)
    vecs = [a[:, s * NT : (s + 1) * NT].T.reshape(D) for s in range(4)]  # t-major
    px2 = a[:, 4 * NT : 5 * NT].sum()
    pxbm = a[:, 5 * NT : 6 * NT].sum()
    cv = a[:, 6 * NT : 7 * NT].sum()
    return vecs, (px2, pxbm, cv)


def combine(stats_per_core):
    """Host epilogue: all-reduce the per-core stat vectors and form the scalar."""
    A = np.zeros(D); Bv = np.zeros(D); Sx = np.zeros(D); Sx2 = np.zeros(D)
    Px2 = Pxbm = Csum = 0.0
    for arr in stats_per_core:
        (a, b, sx, sx2), (px2, pxbm, cv) = _unpack_stats(arr)
        A += a; Bv += b; Sx += sx; Sx2 += sx2
        Px2 += px2; Pxbm += pxbm; Csum += cv
    P = Px2 - 2.0 * Pxbm + Csum                   # sum_i sum_d (x-mu)^2*ie
    neg = A @ Sx2 - 2.0 * (Bv @ Sx) + N * Csum    # sum_{i,j} D[i,j]
    loss = (-0.5 * P + 0.5 / N * neg) / N
    return np.float32(loss)


def run_on_device(in_maps, trace=False, **kwargs):
    from concourse.bass_utils import run_bass_kernel_spmd

    nc = get_program()
    return run_bass_kernel_spmd(nc, in_maps, list(range(NCORES)), trace=trace,
                                **kwargs)


def kernel(x, p_mu, p_logvar):
    in_maps = make_in_maps(x, p_mu, p_logvar)
    br = run_on_device(in_maps)
    return combine([r["stats"] for r in br.results])


# revision 23
# speedup vs baseline: 1.1177x; 1.0210x over previous
"""CLUB loss kernel for Trainium2 (8 NeuronCores, SPMD).

Math
----
Reference computes, with flat_x = transpose(x,(0,2,3,1)).reshape(N,d),
ie = exp(-p_logvar):

  positive[i] = -0.5 * sum_d (x_i - mu_i)^2 * ie_i
  negative[i] = -0.5 * mean_j [ <x_j^2, ie_i> - 2 <x_j, mu_i*ie_i> + <mu_i^2, ie_i> ]
  loss = mean_i (positive - negative)

Because the loss only needs mean_j of a form affine in (x_j, x_j^2), the
(N,N) distance matrix collapses:

  sum_j D[i,j] = <ie_i, Sx2> - 2 <mu_i*ie_i, Sx> + N*<mu_i^2, ie_i>
  with Sx = sum_j x_j, Sx2 = sum_j x_j^2   (d-vectors, global over all rows)

So each core k (rows [784k, 784k+784) == batch element k) reduces its slab to
seven per-channel vectors (free-axis reductions in a channels-on-partitions
layout):

  A    = sum_i ie_i            B    = sum_i mu_i*ie_i
  Sx   = sum_i x_i             Sx2  = sum_i x_i^2
  Px2  = sum_i x_i^2*ie_i      Pxbm = sum_i x_i*mu_i*ie_i
  Cv   = sum_i mu_i^2*ie_i

and the host combines:
  P    = sum(Px2) - 2*sum(Pxbm) + sum(Cv)              # sum_i -2*positive[i]
  neg  = <A, Sx2_g> - 2 <B, Sx_g> + N*sum(Cv_g)        # sum_{i,j} D[i,j]
  loss = (-0.5*P + 0.5/N * neg) / N

Sharding: data-parallel over rows i; x arrives in DRAM already channels-major
per batch element (x[k] is (512, 784) == flat_x-slab transposed), so the
device kernel needs no transposes. mu/logvar slabs are transposed on the host
during input marshalling.
"""

import numpy as np

B, D, H, W = 8, 512, 28, 28
N = B * H * W            # 6272 rows
NCORES = 8
ROWS = N // NCORES       # 784 rows per core == H*W
NT = D // 128            # 4 channel tiles of 128 partitions
NSTAT = 7                # A, B, Sx, Sx2, Px2, Pxbm, Cv

_PROGRAM_CACHE: dict = {}


def build_program():
    """One Bass program, broadcast SPMD to all 8 cores (data differs per core).

    Raw Bass (no TileContext): this walrus build rejects Tile's drain tail
    ("Too many sync wait commands") and the InstTensorTensorReduce encoding
    ("ISA wrong length"), so sync is managed manually and the fused
    multiply+reduce uses scalar_tensor_tensor (which encodes fine).
    """
    from contextlib import ExitStack

    import concourse.bass as bass
    import concourse.mybir as mybir

    f32 = mybir.dt.float32
    Alu = mybir.AluOpType
    Act = mybir.ActivationFunctionType

    nc = bass.Bass()
    xT = nc.declare_dram_parameter("xT", [D, ROWS], f32, isOutput=False)
    muT = nc.declare_dram_parameter("muT", [D, ROWS], f32, isOutput=False)
    lvT = nc.declare_dram_parameter("lvT", [D, ROWS], f32, isOutput=False)
    stats = nc.declare_dram_parameter("stats", [128, NSTAT * NT], f32, isOutput=True)

    with ExitStack() as ctx:
        sb = lambda name, shape: ctx.enter_context(nc.sbuf_tensor(name, shape, f32))
        x = [sb(f"x{t}", [128, ROWS]) for t in range(NT)]
        mu = [sb(f"mu{t}", [128, ROWS]) for t in range(NT)]
        lv = [sb(f"lv{t}", [128, ROWS]) for t in range(NT)]
        ie = [sb(f"ie{t}", [128, ROWS]) for t in range(NT)]
        xs = [sb(f"xs{t}", [128, ROWS]) for t in range(NT)]
        bm = [sb(f"bm{t}", [128, ROWS]) for t in range(NT)]
        wt = sb("wt", [128, 1])              # ACT table warm-up scratch
        sc_act = sb("sc_act", [128, ROWS])   # dead ACT outs (ACT is in-order)
        sc_dve = sb("sc_dve", [128, ROWS])   # dead DVE outs (DVE is in-order)
        st = sb("st", [128, NSTAT * NT])

        sem_x = [ctx.enter_context(nc.semaphore(f"sx{t}")) for t in range(NT)]
        sem_mu = [ctx.enter_context(nc.semaphore(f"sm{t}")) for t in range(NT)]
        sem_lv = [ctx.enter_context(nc.semaphore(f"sl{t}")) for t in range(NT)]
        act_sem = ctx.enter_context(nc.semaphore("act"))
        dve_sem = ctx.enter_context(nc.semaphore("dve"))
        out_sem = ctx.enter_context(nc.semaphore("outs"))
        block = ctx.enter_context(nc.Block())

        def col(s, t):
            c = s * NT + t
            return st[:, c : c + 1]

        @block.sync
        def _(sync):
            # HWDGE input DMAs: cheap issue on the otherwise-idle SP engine.
            # Order lv,x,mu per tile so ACT's exp can start earliest.
            for t in range(NT):
                sl_ = slice(128 * t, 128 * (t + 1))
                sync.dma_start(lv[t][:], lvT[sl_, :]).then_inc(sem_lv[t], 16)
                sync.dma_start(x[t][:], xT[sl_, :]).then_inc(sem_x[t], 16)
                sync.dma_start(mu[t][:], muT[sl_, :]).then_inc(sem_mu[t], 16)
            sync.wait_ge(act_sem, 3 * NT)
            sync.wait_ge(dve_sem, 4 * NT)
            sync.dma_start(stats[:, :], st[:]).then_inc(out_sem, 16)
            sync.wait_ge(out_sem, 16)

        @block.scalar
        def _(scalar):
            # Dummy exp (scale=0): hoists the ACT table load into the DMA wait.
            nc.scalar.activation(wt[:], wt[:], Act.Exp, bias=0.0, scale=0.0)
            for t in range(NT):
                scalar.wait_ge(sem_lv[t], 16)
                # ie = exp(-lv); accum -> A
                nc.scalar.activation(ie[t][:], lv[t][:], Act.Exp, bias=0.0,
                                     scale=-1.0, accum_out=col(0, t)
                                     ).then_inc(act_sem, 1)
                scalar.wait_ge(sem_x[t], 16)
                # xs = x^2; accum -> Sx2
                nc.scalar.activation(xs[t][:], x[t][:], Act.Square,
                                     accum_out=col(3, t)).then_inc(act_sem, 1)
                if t > 0:
                    scalar.wait_ge(act_sem, 3 * t)   # sc_act WAW (self, trivial)
                # copy of x (dead); accum -> Sx
                nc.scalar.activation(sc_act[:], x[t][:], Act.Copy,
                                     accum_out=col(2, t)).then_inc(act_sem, 1)

        @block.vector
        def _(vector):
            for t in range(NT):
                vector.wait_ge(act_sem, 3 * t + 1)   # ie[t] ready
                vector.wait_ge(sem_mu[t], 16)
                # bm = mu*ie; accum -> B
                nc.vector.scalar_tensor_tensor(
                    bm[t][:], mu[t][:], 1.0, ie[t][:], Alu.mult, Alu.mult,
                    accum_out=col(1, t)).then_inc(dve_sem, 1)
                vector.wait_ge(act_sem, 3 * t + 2)   # xs[t] (and x[t]) ready
                if t > 0:
                    vector.wait_ge(dve_sem, 4 * t)   # sc_dve WAW (self, trivial)
                # xs*ie (dead); accum -> Px2
                nc.vector.scalar_tensor_tensor(
                    sc_dve[:], xs[t][:], 1.0, ie[t][:], Alu.mult, Alu.mult,
                    accum_out=col(4, t)).then_inc(dve_sem, 1)
                vector.wait_ge(dve_sem, 4 * t + 2)   # bm[t] RAW + sc_dve WAW (self)
                # x*bm (dead); accum -> Pxbm
                nc.vector.scalar_tensor_tensor(
                    sc_dve[:], x[t][:], 1.0, bm[t][:], Alu.mult, Alu.mult,
                    accum_out=col(5, t)).then_inc(dve_sem, 1)
                vector.wait_ge(dve_sem, 4 * t + 3)   # sc_dve WAW (self)
                # mu*bm (dead); accum -> Cv
                nc.vector.scalar_tensor_tensor(
                    sc_dve[:], mu[t][:], 1.0, bm[t][:], Alu.mult, Alu.mult,
                    accum_out=col(6, t)).then_inc(dve_sem, 1)

    return nc


def get_program():
    if "nc" not in _PROGRAM_CACHE:
        _PROGRAM_CACHE["nc"] = build_program()
    return _PROGRAM_CACHE["nc"]


def make_in_maps(x, p_mu, p_logvar):
    """Shard full inputs into per-core input maps (data-parallel over rows)."""
    x = np.asarray(x, dtype=np.float32)
    p_mu = np.asarray(p_mu, dtype=np.float32)
    p_logvar = np.asarray(p_logvar, dtype=np.float32)
    xk = x.reshape(NCORES, D, ROWS)  # core k's slab of flat_x, transposed
    in_maps = []
    for k in range(NCORES):
        rows = slice(ROWS * k, ROWS * (k + 1))
        in_maps.append({
            "xT": np.ascontiguousarray(xk[k]),
            "muT": np.ascontiguousarray(p_mu[rows].T),
            "lvT": np.ascontiguousarray(p_logvar[rows].T),
        })
    return in_maps


def _unpack_stats(stats_arr):
    """(128, 7*NT) device layout -> (7, 512) per-channel stat vectors."""
    out = np.empty((NSTAT, D), dtype=np.float64)
    for s in range(NSTAT):
        sub = stats_arr[:, s * NT : (s + 1) * NT]  # (128, NT); sub[p, t] = v[t*128+p]
        out[s] = sub.T.reshape(D).astype(np.float64)
    return out


def combine(stats_per_core):
    """Host epilogue: all-reduce the per-core stat vectors and form the scalar."""
    tot = np.zeros((NSTAT, D), dtype=np.float64)
    for arr in stats_per_core:
        tot += _unpack_stats(arr)
    A, Bv, Sx, Sx2, Px2, Pxbm, Cv = tot
    Csum = Cv.sum()
    P = Px2.sum() - 2.0 * Pxbm.sum() + Csum       # sum_i sum_d (x-mu)^2*ie
    neg = A @ Sx2 - 2.0 * (Bv @ Sx) + N * Csum    # sum_{i,j} D[i,j]
    loss = (-0.5 * P + 0.5 / N * neg) / N
    return np.float32(loss)


def run_on_device(in_maps, trace=False, **kwargs):
    from concourse.bass_utils import run_bass_kernel_spmd

    nc = get_program()
    return run_bass_kernel_spmd(nc, in_maps, list(range(NCORES)), trace=trace,
                                **kwargs)


def kernel(x, p_mu, p_logvar):
    in_maps = make_in_maps(x, p_mu, p_logvar)
    br = run_on_device(in_maps)
    return combine([r["stats"] for r in br.results])
